# revision 1
# baseline (speedup 1.0000x reference)
"""Single-head causal attention (B=4, T=4096, C=768, H=64) on 8 NeuronCores.

Sharding: 2 cores per batch. Within a batch the 4096 keys are split between
the two cores by interleaved 128-row blocks (core parity p takes global key
blocks {2g+p}).  Every core computes partial attention (un-normalized
numerator + denominator) for ALL 4096 queries of its batch over ITS 2048
keys; the host adds the two partials and normalizes.  This makes the causal
work exactly equal on all 8 cores and the device program identical (all
core-dependence lives in the input data, including the diagonal masks).

Device program (per core), all matmul operands bf16, accumulation fp32:
  qT  [64, 4096] = Wq^T @ x^T          (x^T supplied pre-transposed, bf16)
  kvT [128,2048] = [Wk|Wv]^T @ x_own^T (own key rows, pre-gathered)
  v   [s,64]     = PE-transpose of vT rows, + ones column -> v' [s,65]
  per q-tile j (512 rows), per own key chunk g<=2j+1 (128 keys):
     scoresT[s,t] = kT_g^T @ qT_j  (PSUM, groups of 2 chunks)
     w = exp(scoresT/8)  (ScalarE, PSUM->SBUF bf16, fused 1/sqrt(64) scale)
     w *= mask          (last two chunks only; mask content is host data)
     outT[65, 512] += v'_g^T @ w  (rows 0..63 numerator^T, row 64 = denom)

Tiles are split fine-grained (per 512-column block) so the Tile scheduler
can overlap input DMA, projections, and attention; all PSUM pools coexist
within the 8 banks so no phase barrier is needed.
"""

import sys

for _p in ("/opt/trn_rl_repo",):
    if _p not in sys.path:
        sys.path.insert(0, _p)

import math
import numpy as np
import ml_dtypes

import concourse.bass as bass
import concourse.mybir as mybir
import concourse.tile as tile
from concourse import bacc
from concourse import bass_utils
from concourse.masks import make_identity

BF16 = mybir.dt.bfloat16
F32 = mybir.dt.float32

P = 128
T = 4096
C = 768
H = 64
CC = C // P        # 6 contraction chunks
OWN = T // 2       # own keys per core
NJ = T // 512      # 8 q-tiles
NCORES = 8

_NC_CACHE = {}


def _build_nc():
    nc = bacc.Bacc("TRN2", target_bir_lowering=False, debug=False,
                   num_devices=NCORES)

    xT = nc.dram_tensor("xT", [C, T], BF16, kind="ExternalInput")
    xTo = nc.dram_tensor("xTo", [C, OWN], BF16, kind="ExternalInput")
    wq = nc.dram_tensor("wq", [P, CC * 64], BF16, kind="ExternalInput")
    wkv = nc.dram_tensor("wkv", [P, CC * 128], BF16, kind="ExternalInput")
    msk = nc.dram_tensor("msk", [P, 1024], BF16, kind="ExternalInput")
    outp = nc.dram_tensor("outp", [NJ, 65, 512], F32, kind="ExternalOutput")

    with tile.TileContext(nc) as tc:
        with (
            tc.tile_pool(name="const", bufs=1) as cst,
            tc.tile_pool(name="big", bufs=1) as big,
            tc.tile_pool(name="pps", bufs=3, space="PSUM") as pps,
            tc.tile_pool(name="sps", bufs=2, space="PSUM") as sps_pool,
            tc.tile_pool(name="ops", bufs=1, space="PSUM") as ops_pool,
            tc.tile_pool(name="wt", bufs=3) as wt_pool,
            tc.tile_pool(name="osb", bufs=4) as osb_pool,
        ):
            ident = cst.tile([P, P], BF16)
            make_identity(nc, ident[:])
            wq_sb = cst.tile([P, CC * 64], BF16)
            nc.sync.dma_start(wq_sb[:], wq[:])
            wkv_sb = cst.tile([P, CC * 128], BF16)
            nc.sync.dma_start(wkv_sb[:], wkv[:])
            msk_sb = cst.tile([P, 1024], BF16)
            nc.sync.dma_start(msk_sb[:], msk[:])

            # Head loads (small, duplicated columns) unblock kv-block 0 and
            # q-blocks 0/1 within a few us; the bulk arrives as large halves.
            xto_half, xt_half = [], []
            for h in range(2):
                for i in range(CC):
                    t_ = big.tile([P, OWN // 2], BF16, tag=f"xto{i}_{h}")
                    nc.sync.dma_start(t_[:], xTo[P * i:P * (i + 1),
                                                 (OWN // 2) * h:(OWN // 2) * (h + 1)])
                    xto_half.append(t_)  # xto_half[h*CC + i]
                for i in range(CC):
                    t_ = big.tile([P, OWN], BF16, tag=f"xt{i}_{h}")
                    nc.sync.dma_start(t_[:], xT[P * i:P * (i + 1),
                                                OWN * h:OWN * (h + 1)])
                    xt_half.append(t_)  # xt_half[h*CC + i]

            def xto_slice(ci, tb):
                h, r = tb // 2, tb % 2
                return xto_half[h * CC + ci][:, 512 * r:512 * (r + 1)]

            def xt_slice(ci, j):
                h, r = j // 4, j % 4
                return xt_half[h * CC + ci][:, 512 * r:512 * (r + 1)]

            kvts = []
            vsbs = []

            def emit_kv_block(tb):
                ps = pps.tile([P, 512], F32, tag="pps")
                for ci in range(CC):
                    nc.tensor.matmul(
                        ps[:], wkv_sb[:, 128 * ci:128 * (ci + 1)],
                        xto_slice(ci, tb),
                        start=(ci == 0), stop=(ci == CC - 1))
                kvt = big.tile([P, 512], BF16, tag=f"kvT{tb}")
                nc.vector.tensor_copy(kvt[:], ps[:])
                kvts.append(kvt)
                # v' tiles for the 4 chunks of this block
                vsb = big.tile([P, 4 * 65], BF16, tag=f"v{tb}")
                nc.vector.memset(vsb[:], 1.0)
                vp = pps.tile([P, 512], BF16, tag="pps")
                for i in range(4):
                    nc.tensor.transpose(
                        vp[:, 64 * i:64 * (i + 1)],
                        kvt[64:128, 128 * i:128 * (i + 1)],
                        ident[64:128, 64:128])
                for i in range(4):
                    nc.vector.tensor_copy(vsb[:, 65 * i:65 * i + 64],
                                          vp[:, 64 * i:64 * (i + 1)])
                vsbs.append(vsb)

            for j in range(NJ):
                if j % 2 == 0:
                    emit_kv_block(j // 2)
                ps = pps.tile([64, 512], F32, tag="pps")
                for ci in range(CC):
                    nc.tensor.matmul(
                        ps[:], wq_sb[:, 64 * ci:64 * (ci + 1)],
                        xt_slice(ci, j),
                        start=(ci == 0), stop=(ci == CC - 1))
                qt = big.tile([64, 512], BF16, tag=f"qT{j}")
                nc.vector.tensor_copy(qt[:], ps[:])

                nchunks = 2 * j + 2
                ops = ops_pool.tile([65, 512], F32, tag="ops")
                g = 0
                while g < nchunks:
                    gn = min(2, nchunks - g)
                    sp = sps_pool.tile([P, 512 * gn], F32, tag="sps")
                    for i in range(gn):
                        gg = g + i
                        nc.tensor.matmul(
                            sp[:, 512 * i:512 * (i + 1)],
                            kvts[gg // 4][0:64, 128 * (gg % 4):128 * (gg % 4 + 1)],
                            qt[:], start=True, stop=True)
                    wt = wt_pool.tile([P, 512 * gn], BF16, tag="wt")
                    nc.scalar.activation(
                        wt[:], sp[:], mybir.ActivationFunctionType.Exp,
                        scale=1.0 / math.sqrt(H))
                    for i in range(gn):
                        gg = g + i
                        if gg == 2 * j:
                            nc.vector.tensor_mul(
                                wt[:, 512 * i:512 * (i + 1)],
                                wt[:, 512 * i:512 * (i + 1)],
                                msk_sb[:, 0:512])
                        elif gg == 2 * j + 1:
                            nc.vector.tensor_mul(
                                wt[:, 512 * i:512 * (i + 1)],
                                wt[:, 512 * i:512 * (i + 1)],
                                msk_sb[:, 512:1024])
                    for i in range(gn):
                        gg = g + i
                        nc.tensor.matmul(
                            ops[:],
                            vsbs[gg // 4][:, 65 * (gg % 4):65 * (gg % 4 + 1)],
                            wt[:, 512 * i:512 * (i + 1)],
                            start=(gg == 0), stop=(gg == nchunks - 1))
                    g += gn
                osb = osb_pool.tile([65, 512], F32, tag="osb")
                nc.vector.tensor_copy(osb[:], ops[:])
                nc.sync.dma_start(outp[j], osb[:])

    nc.compile()
    return nc


def get_nc():
    if "nc" not in _NC_CACHE:
        _NC_CACHE["nc"] = _build_nc()
    return _NC_CACHE["nc"]


def make_in_maps(x, Wq, Wk, Wv):
    bf = ml_dtypes.bfloat16
    wq_in = np.zeros((P, CC * 64), bf)
    wkv_in = np.zeros((P, CC * 128), bf)
    for ci in range(CC):
        wq_in[:, 64 * ci:64 * (ci + 1)] = Wq[P * ci:P * (ci + 1), :].astype(bf)
        wkv_in[:, 128 * ci:128 * ci + 64] = Wk[P * ci:P * (ci + 1), :].astype(bf)
        wkv_in[:, 128 * ci + 64:128 * (ci + 1)] = Wv[P * ci:P * (ci + 1), :].astype(bf)
    si = np.arange(P)[:, None]
    ti = np.arange(512)[None, :]
    in_maps = []
    rows = np.arange(T)
    for c in range(NCORES):
        b, p = c // 2, c % 2
        xb = np.asarray(x[b], dtype=np.float32)
        xT_all = np.ascontiguousarray(xb.T).astype(bf)
        own = rows[(rows // P) % 2 == p]
        xTo_in = np.ascontiguousarray(xb[own].T).astype(bf)
        m0 = ((si + P * p) <= ti).astype(bf)
        m1 = ((si + 256 + P * p) <= ti).astype(bf)
        msk_in = np.ascontiguousarray(np.concatenate([m0, m1], axis=1))
        in_maps.append({"xT": xT_all, "xTo": xTo_in, "wq": wq_in,
                        "wkv": wkv_in, "msk": msk_in})
    return in_maps


def combine(results, B=4):
    out = np.zeros((B, T, H), np.float32)
    for b in range(B):
        o = results[2 * b]["outp"].astype(np.float32) \
            + results[2 * b + 1]["outp"].astype(np.float32)
        num = o[:, :64, :]                 # [NJ, 64, 512]
        den = o[:, 64, :]                  # [NJ, 512]
        ob = num / den[:, None, :]
        out[b] = ob.transpose(0, 2, 1).reshape(T, H)
    return out


def kernel(x, Wq, Wk, Wv, **run_kwargs):
    nc = get_nc()
    in_maps = make_in_maps(x, Wq, Wk, Wv)
    res = bass_utils.run_bass_kernel_spmd(nc, in_maps,
                                          list(range(NCORES)), **run_kwargs)
    out = combine(res.results, B=x.shape[0])
    if run_kwargs:
        kernel.last_results = res
    return out



# revision 9
# speedup vs baseline: 1.0500x; 1.0500x over previous
"""Single-head causal attention (B=4, T=4096, C=768, H=64) on 8 NeuronCores.

Sharding: 2 cores per batch; core parity p owns the interleaved 128-row key
blocks {2g+p}.  Every core computes partial attention (unnormalized numerator
+ denominator) for ALL 4096 queries over ITS 2048 keys; the host adds the two
partials and normalizes.  The causal work is exactly equal on all 8 cores and
the device program is identical: all core-dependence lives in input data.
For odd-parity cores the xT tensor is stored with adjacent 128-column blocks
swapped, so the program's fixed even-block kv slices read the odd key blocks;
queries come out block-permuted, which the masks and the host combine undo.

Device program highlights (vs the plain bf16 version):
  * out-matmul is transposed: out[128q, 65] += wt_chunk^T @ v'_chunk, using
    the full 128x128 PE array (65 moving rows per chunk instead of 512).
  * scores for q-tiles j>=JBF run as fp8e4m3 DoubleRow matmuls (half cost);
    q/k are quantized to fp8 with a x16 weight pre-scale (fp8 subnormal
    avoidance), and the DR second k-subtile is a zero plane.  Early tiles
    stay bf16 because short softmax rows don't average away fp8 noise.
  * kv projection slices the own-key columns straight out of the full xT
    tile (no separate xTo load); v' is built by PE transpose.
  * exp runs on big fused Activation instructions ([128,1536]/[128,1024]
    PSUM groups); diagonal masks are bf16 multiplies on the Pool engine.
"""

import sys

for _p in ("/opt/trn_rl_repo",):
    if _p not in sys.path:
        sys.path.insert(0, _p)

import math
import numpy as np
import ml_dtypes

import concourse.bass as bass
import concourse.mybir as mybir
import concourse.tile as tile
from concourse import bacc
from concourse import bass_utils
from concourse.masks import make_identity

BF16 = mybir.dt.bfloat16
FP8 = mybir.dt.float8e4
F32 = mybir.dt.float32

P = 128
T = 4096
C = 768
H = 64
CC = C // P        # 6 contraction chunks
NJ = T // 512      # 8 q-tiles
NCORES = 8
WSCALE = 16.0      # weight pre-scale for fp8 q/k
JBF = 3            # q-tiles < JBF use bf16 scores
EXP_SCALE = 1.0 / (WSCALE * WSCALE * math.sqrt(H))

_NC_CACHE = {}


def _build_nc():
    nc = bacc.Bacc("TRN2", target_bir_lowering=False, debug=False,
                   num_devices=NCORES)

    xT = nc.dram_tensor("xT", [P, CC * T], BF16, kind="ExternalInput")
    wq = nc.dram_tensor("wq", [P, CC * 64], BF16, kind="ExternalInput")
    wkv = nc.dram_tensor("wkv", [P, CC * 128], BF16, kind="ExternalInput")
    msk = nc.dram_tensor("msk", [P, 1024], BF16, kind="ExternalInput")
    outp = nc.dram_tensor("outp", [NJ, P, 260], BF16, kind="ExternalOutput")

    with tile.TileContext(nc) as tc:
        with (
            tc.tile_pool(name="const", bufs=1) as cst,
            tc.tile_pool(name="big", bufs=1) as big,
            tc.tile_pool(name="spsA", bufs=1, space="PSUM") as spsA,
            tc.tile_pool(name="spsB", bufs=1, space="PSUM") as spsB,
            tc.tile_pool(name="pps", bufs=2, space="PSUM") as pps,
            tc.tile_pool(name="oac", bufs=1, space="PSUM") as oac,
            tc.tile_pool(name="wt", bufs=3) as wt_pool,
            tc.tile_pool(name="osb", bufs=2) as osb_pool,
        ):
            ident = cst.tile([P, P], BF16)
            make_identity(nc, ident[:])
            wq_sb = cst.tile([P, CC, 64], BF16)
            nc.sync.dma_start(wq_sb[:], wq[:].rearrange("p (c h) -> p c h", c=CC))
            wkv_sb = cst.tile([P, CC, 128], BF16)
            nc.sync.dma_start(wkv_sb[:], wkv[:].rearrange("p (c h) -> p c h", c=CC))
            msk_sb = cst.tile([P, 1024], BF16)
            nc.sync.dma_start(msk_sb[:], msk[:])

            # Full xT in SBUF, ci-major.  Column spans sized so early q/kv
            # tiles unblock quickly while later spans amortize DMA overhead.
            xsb = big.tile([P, CC, T], BF16, tag="xsb")
            spans = [(0, 512), (512, 1024), (1024, 2048),
                     (2048, 3072), (3072, 4096)]
            for lo, hi in spans:
                for ci in range(CC):
                    nc.sync.dma_start(xsb[:, ci, lo:hi],
                                      xT[:, ci * T + lo: ci * T + hi])

            qsb = {}   # fp8 [64, 2, 512] per j (slot1 zero)
            qtb = {}   # bf16 [64, 512] for j < JBF
            kt8 = []   # fp8 [64, 2, 512] per key block (slot1 zero)
            kvt = []   # bf16 [128, 512] per key block (kT | vT)
            vsb = []   # bf16 [128, 4, 65] per key block (v' with ones col)

            def emit_kv_block(blk):
                kvp = pps.tile([P, 512], F32, tag="pps")
                for g4 in range(4):
                    base = P * (8 * blk + 2 * g4)  # parity handled by data
                    for ci in range(CC):
                        # one start per PSUM bank: start marks the whole 2KB
                        # bank pending-zero, so later regions must not re-start
                        nc.tensor.matmul(
                            kvp[:, 128 * g4:128 * (g4 + 1)],
                            wkv_sb[:, ci, :],
                            xsb[:, ci, base:base + 128],
                            start=(ci == 0 and g4 == 0), stop=(ci == CC - 1),
                            skip_group_check=True)
                kv_t = big.tile([P, 512], BF16, tag=f"kvt{blk}")
                nc.vector.tensor_copy(kv_t[:], kvp[:])
                kvt.append(kv_t)
                k8 = big.tile([64, 2, 512], FP8, tag=f"kt8{blk}")
                nc.vector.memset(k8[:, 1, :], 0.0)
                nc.vector.tensor_copy(k8[:, 0, :], kvp[0:64, :])
                kt8.append(k8)
                # v' tiles: PE-transpose the vT rows
                vp = pps.tile([P, 256], BF16, tag="pps")
                for g4 in range(4):
                    nc.tensor.transpose(
                        vp[:, 64 * g4:64 * (g4 + 1)],
                        kv_t[64:128, 128 * g4:128 * (g4 + 1)],
                        ident[64:128, 64:128])
                vs = big.tile([P, 4, 65], BF16, tag=f"vsb{blk}")
                nc.vector.memset(vs[:], 1.0)
                for g4 in range(4):
                    nc.vector.tensor_copy(vs[:, g4, 0:64],
                                          vp[:, 64 * g4:64 * (g4 + 1)])
                vsb.append(vs)

            toggle = [0]  # alternates spsA / spsB

            for j in range(NJ):
                if j % 2 == 0:
                    emit_kv_block(j // 2)
                # q projection for this 512-query tile
                qp = pps.tile([64, 512], F32, tag="pps")
                for ci in range(CC):
                    nc.tensor.matmul(
                        qp[:], wq_sb[:, ci, :],
                        xsb[:, ci, 512 * j:512 * (j + 1)],
                        start=(ci == 0), stop=(ci == CC - 1))
                q8 = big.tile([64, 2, 512], FP8, tag=f"q8{j}")
                nc.vector.memset(q8[:, 1, :], 0.0)
                nc.vector.tensor_copy(q8[:, 0, :], qp[:])
                qsb[j] = q8
                if j < JBF:
                    qt = big.tile([64, 512], BF16, tag=f"qt{j}")
                    nc.vector.tensor_copy(qt[:], qp[:])
                    qtb[j] = qt

                nchunks = 2 * j + 2
                ot = oac.tile([P, 4, 65], F32, tag="oac")
                g = 0
                while g < nchunks:
                    if toggle[0] == 0:
                        gn = min(3, nchunks - g)
                        sp = spsA.tile([P, 512 * gn], F32, tag="spsA")
                    else:
                        gn = min(2, nchunks - g)
                        sp = spsB.tile([P, 512 * gn], F32, tag="spsB")
                    toggle[0] ^= 1
                    for i in range(gn):
                        gg = g + i
                        blk_g, sub = gg // 4, gg % 4
                        if j < JBF:
                            nc.tensor.matmul(
                                sp[:, 512 * i:512 * (i + 1)],
                                kvt[blk_g][0:64, 128 * sub:128 * (sub + 1)],
                                qtb[j][:], start=True, stop=True)
                        else:
                            nc.tensor.matmul(
                                sp[:, 512 * i:512 * (i + 1)],
                                kt8[blk_g][:, :, 128 * sub:128 * (sub + 1)],
                                qsb[j][:],
                                start=True, stop=True,
                                perf_mode=mybir.MatmulPerfMode.DoubleRow)
                    wt = wt_pool.tile([P, 512 * gn], BF16, tag="wt")
                    nc.scalar.activation(
                        wt[:], sp[:], mybir.ActivationFunctionType.Exp,
                        scale=EXP_SCALE)
                    for i in range(gn):
                        gg = g + i
                        if gg == 2 * j:
                            nc.vector.tensor_mul(
                                wt[:, 512 * i:512 * (i + 1)],
                                wt[:, 512 * i:512 * (i + 1)],
                                msk_sb[:, 0:512])
                        elif gg == 2 * j + 1:
                            nc.vector.tensor_mul(
                                wt[:, 512 * i:512 * (i + 1)],
                                wt[:, 512 * i:512 * (i + 1)],
                                msk_sb[:, 512:1024])
                    for i in range(gn):
                        gg = g + i
                        blk_g, sub = gg // 4, gg % 4
                        for r in range(4):
                            nc.tensor.matmul(
                                ot[:, r, :],
                                wt[:, 512 * i + 128 * r:512 * i + 128 * (r + 1)],
                                vsb[blk_g][:, sub, :],
                                start=(gg == 0 and r == 0),
                                stop=(gg == nchunks - 1),
                                skip_group_check=True)
                    g += gn
                osb = osb_pool.tile([P, 4, 65], BF16, tag="osb")
                nc.vector.tensor_copy(osb[:], ot[:])
                nc.sync.dma_start(outp[j], osb[:])

    nc.compile()
    return nc


def get_nc():
    if "nc" not in _NC_CACHE:
        _NC_CACHE["nc"] = _build_nc()
    return _NC_CACHE["nc"]


def _masks(p):
    """Masks for the two diagonal chunks, in STORED query coordinates.

    Own-key chunk g=2j sits at within-tile key offset 128*1 for p=1 (stored
    block-swap) and 128*0 for p=0; chunk g=2j+1 at 128*3 (p=1) / 128*2 (p=0).
    Stored query subcol r maps to global within-tile block r^p.
    """
    bf = ml_dtypes.bfloat16
    s = np.arange(P)[:, None]
    t = np.arange(512)[None, :]
    t128 = t % 128
    qb = (t // 128) ^ p              # global query block within tile
    kb0 = p                          # within-tile key block of chunk 2j
    kb1 = 2 + p                      # within-tile key block of chunk 2j+1
    m0 = ((kb0 * 128 + s) <= (qb * 128 + t128)).astype(bf)
    m1 = ((kb1 * 128 + s) <= (qb * 128 + t128)).astype(bf)
    return np.ascontiguousarray(np.concatenate([m0, m1], axis=1))


def make_in_maps(x, Wq, Wk, Wv):
    bf = ml_dtypes.bfloat16
    wq_in = np.zeros((P, CC * 64), bf)
    wkv_in = np.zeros((P, CC * 128), bf)
    for ci in range(CC):
        wq_in[:, 64 * ci:64 * (ci + 1)] = \
            (Wq[P * ci:P * (ci + 1), :] * WSCALE).astype(bf)
        wkv_in[:, 128 * ci:128 * ci + 64] = \
            (Wk[P * ci:P * (ci + 1), :] * WSCALE).astype(bf)
        wkv_in[:, 128 * ci + 64:128 * (ci + 1)] = \
            Wv[P * ci:P * (ci + 1), :].astype(bf)
    in_maps = []
    for c in range(NCORES):
        b, p = c // 2, c % 2
        xb = np.asarray(x[b], dtype=np.float32)       # [T, C]
        if p == 1:
            xb = xb.reshape(T // 256, 2, 128, C)[:, ::-1].reshape(T, C)
        xT_all = np.ascontiguousarray(
            xb.T.reshape(CC, P, T).transpose(1, 0, 2).reshape(P, CC * T)
        ).astype(bf)
        in_maps.append({"xT": xT_all, "wq": wq_in, "wkv": wkv_in,
                        "msk": _masks(p)})
    return in_maps


def combine(results, B=4):
    out = np.zeros((B, T, H), np.float32)
    for b in range(B):
        o0 = results[2 * b]["outp"].astype(np.float32).reshape(NJ, P, 4, 65)
        o1 = results[2 * b + 1]["outp"].astype(np.float32).reshape(NJ, P, 4, 65)
        o1 = o1[:, :, [1, 0, 3, 2], :]        # undo stored block swap
        o = o0 + o1
        num = o[..., :64]
        den = o[..., 64]
        ob = num / den[..., None]              # [NJ, 128, 4, 64]
        out[b] = ob.transpose(0, 2, 1, 3).reshape(T, H)
    return out


def kernel(x, Wq, Wk, Wv, **run_kwargs):
    nc = get_nc()
    in_maps = make_in_maps(x, Wq, Wk, Wv)
    res = bass_utils.run_bass_kernel_spmd(nc, in_maps,
                                          list(range(NCORES)), **run_kwargs)
    out = combine(res.results, B=x.shape[0])
    if run_kwargs:
        kernel.last_results = res
    return out
